# revision 3
# baseline (speedup 1.0000x reference)
"""Trainium2 Bass kernel for nn_Attention (B=2, N=4096, DIM=768, H=12 heads).

Sharding: 24 (batch, head) pairs over 8 cores -> 3 heads per core, 4 cores
per batch element. Each core computes, for its batch b and its 3 heads:
  q,k,v projections -> full attention (flash-style, no score materialization
  to HBM) -> partial output projection  y_partial^T = sum_h wp_h^T @ O_h^T.
The host sums the 4 partials per batch and adds the bias.

Device dataflow (all matmul inputs bf16, fp32 PSUM accumulation):
  - host passes x^T, w_qk^T, w_v^T, w_p^T pre-transposed/pre-sliced in bf16
  - qk^T = [w_q|w_k]^T.T @ x^T    -> q^T,k^T [64, 4096] per head (d-major)
  - S^T[k,q] = k^T.T @ q^T        -> PSUM, 2-way row-packed (K=64)
  - P^T = exp(S^T * scale)        -> ScalarE (the bottleneck engine)
  - O^T|den = [V|1].T @ P^T       -> PSUM accumulate over k blocks
  - O^T /= den (recip + gpsimd partition-broadcast + DVE mult)
  - y^T += wp_h^T.T @ O_h^T       -> per-head K=64 accumulation
"""

import numpy as np
import ml_dtypes

import concourse.bacc as bacc
import concourse.mybir as mybir
import concourse.tile as tile
from concourse.bass_utils import run_bass_kernel_spmd

BF16 = mybir.dt.bfloat16
F32 = mybir.dt.float32

DIM = 768
N = 4096
NUM_HEADS = 12
HEAD_DIM = 64
SCALE = HEAD_DIM ** -0.5
B = 2
NCORES = 8
HPC = 3  # heads per core
CCH = DIM // 128  # 6 contraction chunks of 128
NQT = 8  # q tiles of 512
QT = 512
NKB = 32  # k blocks of 128
KB = 128
GRP = 3  # k-blocks per exp group (3 PSUM banks)


def build_program():
    nc = bacc.Bacc("TRN2", target_bir_lowering=False, debug=False)

    xT = nc.dram_tensor("xT", [DIM, N], BF16, kind="ExternalInput")
    wqkT = nc.dram_tensor("wqkT", [DIM, HPC * 128], BF16, kind="ExternalInput")
    wvT = nc.dram_tensor("wvT", [DIM, HPC * 64], BF16, kind="ExternalInput")
    wpT = nc.dram_tensor("wpT", [HPC * 64, DIM], BF16, kind="ExternalInput")
    yT = nc.dram_tensor("yT", [DIM, N], F32, kind="ExternalOutput")

    with tile.TileContext(nc) as tc:
        with (
            tc.tile_pool(name="wpool", bufs=1) as wpool,
            tc.tile_pool(name="qkpool", bufs=1) as qkpool,
            tc.tile_pool(name="pspool", bufs=2, space="PSUM") as pspool,
            tc.tile_pool(name="accpool", bufs=1, space="PSUM") as accpool,
            tc.tile_pool(name="espool", bufs=3) as espool,
            tc.tile_pool(name="dpool", bufs=3) as dpool,
            tc.tile_pool(name="opool", bufs=6) as opool,
            tc.tile_pool(name="ypool", bufs=4) as ypool,
        ):
            xT_sb = wpool.tile([128, CCH * N], BF16, tag="xT")
            wqk_sb = wpool.tile([128, CCH * HPC * 128], BF16, tag="wqk")
            wv_sb = wpool.tile([128, CCH * HPC * 64], BF16, tag="wv")
            wp_sb = wpool.tile([64, HPC * DIM], BF16, tag="wp")
            # per head: cols 0:4096 k^T (both partition halves), 4096:8192 q^T
            T = [
                qkpool.tile([128, 2 * N], BF16, tag=f"T{h}", name=f"T{h}")
                for h in range(HPC)
            ]
            # per head: 32 blocks of [128 tok, 64 v | 1 ones]
            V = [
                qkpool.tile([128, NKB * 65], BF16, tag=f"V{h}", name=f"V{h}")
                for h in range(HPC)
            ]

            for c in range(CCH):
                nc.sync.dma_start(
                    out=xT_sb[:, c * N:(c + 1) * N], in_=xT[c * 128:(c + 1) * 128, :]
                )
                nc.sync.dma_start(
                    out=wqk_sb[:, c * 384:(c + 1) * 384],
                    in_=wqkT[c * 128:(c + 1) * 128, :],
                )
                nc.sync.dma_start(
                    out=wv_sb[:, c * 192:(c + 1) * 192],
                    in_=wvT[c * 128:(c + 1) * 128, :],
                )
            for h in range(HPC):
                nc.sync.dma_start(
                    out=wp_sb[0:64, h * DIM:(h + 1) * DIM],
                    in_=wpT[h * 64:(h + 1) * 64, :],
                )
                nc.gpsimd.memset(V[h][:], 1.0)

            # ---- Phase A: qkv projections ----
            def emit_qk_head(h):
                for qt in range(NQT):
                    ps = pspool.tile([128, QT], F32, tag="s")
                    for c in range(CCH):
                        nc.tensor.matmul(
                            ps[:],
                            lhsT=wqk_sb[:, c * 384 + h * 128: c * 384 + (h + 1) * 128],
                            rhs=xT_sb[:, c * N + qt * QT: c * N + (qt + 1) * QT],
                            start=(c == 0),
                            stop=(c == CCH - 1),
                        )
                    # q rows 0:64 -> q^T lo copy; k rows 64:128 -> k^T hi copy
                    nc.vector.tensor_copy(
                        T[h][0:64, N + qt * QT: N + (qt + 1) * QT], ps[0:64, :]
                    )
                    nc.vector.tensor_copy(
                        T[h][64:128, qt * QT:(qt + 1) * QT], ps[64:128, :]
                    )
                # duplicate to the other partition half (cross-partition: DMA)
                nc.sync.dma_start(out=T[h][0:64, 0:N], in_=T[h][64:128, 0:N])
                nc.sync.dma_start(out=T[h][64:128, N:2 * N], in_=T[h][0:64, N:2 * N])

            def emit_v_all():
                for tt in range(NKB):
                    ps = pspool.tile([128, HPC * 64], F32, tag="s")
                    for c in range(CCH):
                        nc.tensor.matmul(
                            ps[:],
                            lhsT=xT_sb[:, c * N + tt * 128: c * N + tt * 128 + 128],
                            rhs=wv_sb[:, c * 192:(c + 1) * 192],
                            start=(c == 0),
                            stop=(c == CCH - 1),
                        )
                    for h in range(HPC):
                        nc.vector.tensor_copy(
                            V[h][:, tt * 65: tt * 65 + 64],
                            ps[:, h * 64:(h + 1) * 64],
                        )

            emit_qk_head(0)
            emit_v_all()
            emit_qk_head(1)
            emit_qk_head(2)

            # ---- Phase B: attention + projection, q-tile major ----
            ngrp = (NKB + GRP - 1) // GRP
            for qt in range(NQT):
                O = []
                for h in range(HPC):
                    po = accpool.tile([65, QT], F32, tag="o")
                    for g in range(ngrp):
                        nkb = min(GRP, NKB - g * GRP)
                        ps = pspool.tile([128, nkb * QT], F32, tag="s")
                        es = espool.tile([128, nkb * QT], BF16, tag="es")
                        for j in range(nkb):
                            kb = g * GRP + j
                            o = 64 * (kb % 2)
                            nc.tensor.matmul(
                                ps[:, j * QT:(j + 1) * QT],
                                lhsT=T[h][o:o + 64, kb * KB:(kb + 1) * KB],
                                rhs=T[h][o:o + 64, N + qt * QT: N + (qt + 1) * QT],
                                start=True,
                                stop=True,
                            )
                        nc.scalar.activation(
                            es[:], ps[:], mybir.ActivationFunctionType.Exp, scale=SCALE
                        )
                        for j in range(nkb):
                            kb = g * GRP + j
                            nc.tensor.matmul(
                                po[:],
                                lhsT=V[h][:, kb * 65: kb * 65 + 65],
                                rhs=es[:, j * QT:(j + 1) * QT],
                                start=(kb == 0),
                                stop=(kb == NKB - 1),
                                skip_group_check=True,
                            )
                    # normalize: O^T[0:64] / den(row 64)
                    # NB: reciprocal_approx_fast misreads PSUM sources on HW —
                    # copy the denominator row to SBUF first.
                    dr0 = dpool.tile([1, QT], F32, tag="dr0")
                    nc.vector.tensor_copy(dr0[:], po[64:65, :])
                    dr = dpool.tile([1, QT], F32, tag="dr")
                    nc.vector.reciprocal_approx_fast(out=dr[:], in_=dr0[:])
                    db = dpool.tile([64, QT], F32, tag="db")
                    nc.gpsimd.partition_broadcast(db[:], dr[:])
                    oh = opool.tile([64, QT], BF16, tag="O")
                    nc.vector.tensor_mul(oh[:], po[0:64, :], db[:])
                    O.append(oh)
                # projection for this q tile
                for oc in range(CCH):
                    py = accpool.tile([128, QT], F32, tag="yb")
                    for h in range(HPC):
                        nc.tensor.matmul(
                            py[:],
                            lhsT=wp_sb[0:64, h * DIM + oc * 128: h * DIM + (oc + 1) * 128],
                            rhs=O[h][:],
                            start=(h == 0),
                            stop=(h == HPC - 1),
                        )
                    ysb = ypool.tile([128, QT], F32, tag="y")
                    nc.vector.tensor_copy(ysb[:], py[:])
                    nc.sync.dma_start(
                        out=yT[oc * 128:(oc + 1) * 128, qt * QT:(qt + 1) * QT],
                        in_=ysb[:],
                    )

    nc.compile()
    return nc


def make_in_maps(x, w_qkv):
    """Build the 8 per-core input maps from the full fp32 inputs."""
    bf = ml_dtypes.bfloat16
    in_maps = []
    for core in range(NCORES):
        b = core // 4
        hs = [(core % 4) * HPC + i for i in range(HPC)]
        xTb = np.ascontiguousarray(np.asarray(x[b]).T).astype(bf)
        wqk = np.empty((DIM, HPC * 128), dtype=bf)
        wv = np.empty((DIM, HPC * 64), dtype=bf)
        for i, h in enumerate(hs):
            wqk[:, i * 128: i * 128 + 64] = w_qkv[h * 64:(h + 1) * 64, :].T
            wqk[:, i * 128 + 64: i * 128 + 128] = w_qkv[DIM + h * 64: DIM + (h + 1) * 64, :].T
            wv[:, i * 64:(i + 1) * 64] = w_qkv[2 * DIM + h * 64: 2 * DIM + (h + 1) * 64, :].T
        in_maps.append({"xT": xTb, "wqkT": wqk, "wvT": wv})
    return in_maps


def make_wp_map(core, w_proj):
    bf = ml_dtypes.bfloat16
    hs = [(core % 4) * HPC + i for i in range(HPC)]
    wp = np.empty((HPC * 64, DIM), dtype=bf)
    for i, h in enumerate(hs):
        wp[i * 64:(i + 1) * 64, :] = w_proj[:, h * 64:(h + 1) * 64].T
    return wp


_NC = None


def kernel(x, w_qkv, w_proj, b_proj):
    global _NC
    if _NC is None:
        _NC = build_program()
    x = np.asarray(x, dtype=np.float32)
    w_qkv = np.asarray(w_qkv, dtype=np.float32)
    w_proj = np.asarray(w_proj, dtype=np.float32)
    b_proj = np.asarray(b_proj, dtype=np.float32)

    in_maps = make_in_maps(x, w_qkv)
    for core in range(NCORES):
        in_maps[core]["wpT"] = make_wp_map(core, w_proj)

    r = run_bass_kernel_spmd(_NC, in_maps, list(range(NCORES)))
    y = np.zeros((B, N, DIM), dtype=np.float32)
    for core in range(NCORES):
        b = core // 4
        y[b] += r.results[core]["yT"].T
    y += b_proj[None, None, :]
    return y


# revision 11
# speedup vs baseline: 1.0019x; 1.0019x over previous
"""Trainium2 Bass kernel for nn_Attention (B=2, N=4096, DIM=768, H=12 heads).

Sharding: 24 (batch, head) pairs over 8 cores -> 3 heads per core, 4 cores
per batch element. Each core computes, for its batch b and its 3 heads:
  q,k,v projections -> full attention (flash-style, no score materialization
  to HBM) -> partial output projection  y_partial^T = sum_h wp_h^T @ O_h^T.
The host sums the 4 partials per batch and adds the bias.

Device dataflow (all matmul inputs bf16, fp32 PSUM accumulation):
  - host passes x^T, w_qk^T, w_v^T, w_p^T pre-transposed/pre-sliced in bf16
  - qk^T = [w_q|w_k]^T.T @ x^T    -> q^T,k^T [64, 4096] per head (d-major)
  - S^T[k,q] = k^T.T @ q^T        -> PSUM, 2-way row-packed (K=64)
  - P^T = exp(S^T * scale)        -> ScalarE (the bottleneck engine)
  - O^T|den = [V|1].T @ P^T       -> PSUM accumulate over k blocks
  - O^T /= den (recip + gpsimd partition-broadcast + DVE mult)
  - y^T += wp_h^T.T @ O_h^T       -> per-head K=64 accumulation
"""

import numpy as np
import ml_dtypes

import concourse.bacc as bacc
import concourse.mybir as mybir
import concourse.tile as tile
from concourse.bass_utils import run_bass_kernel_spmd

BF16 = mybir.dt.bfloat16
F32 = mybir.dt.float32

DIM = 768
N = 4096
NUM_HEADS = 12
HEAD_DIM = 64
SCALE = HEAD_DIM ** -0.5
B = 2
NCORES = 8
HPC = 3  # heads per core
CCH = DIM // 128  # 6 contraction chunks of 128
NQT = 8  # q tiles of 512
QT = 512
NKB = 32  # k blocks of 128
KB = 128
GRP = 2  # k-blocks per exp group (2 PSUM banks, aligns with row-pack pairs)


def build_program():
    nc = bacc.Bacc("TRN2", target_bir_lowering=False, debug=False)

    xT = nc.dram_tensor("xT", [DIM, N], BF16, kind="ExternalInput")
    wqkT = nc.dram_tensor("wqkT", [DIM, HPC * 128], BF16, kind="ExternalInput")
    wvT = nc.dram_tensor("wvT", [DIM, HPC * 64], BF16, kind="ExternalInput")
    wpT = nc.dram_tensor("wpT", [HPC * 64, DIM], BF16, kind="ExternalInput")
    yT = nc.dram_tensor("yT", [DIM, N], F32, kind="ExternalOutput")

    with tile.TileContext(nc) as tc:
        with (
            tc.tile_pool(name="wpool", bufs=1) as wpool,
            tc.tile_pool(name="qkpool", bufs=1) as qkpool,
            tc.tile_pool(name="pspool", bufs=2, space="PSUM") as pspool,
            tc.tile_pool(name="vpool", bufs=1, space="PSUM") as vpool,
            tc.tile_pool(name="accpool", bufs=1, space="PSUM") as accpool,
            tc.tile_pool(name="espool", bufs=8) as espool,
            tc.tile_pool(name="dpool", bufs=3) as dpool,
            tc.tile_pool(name="opool", bufs=26) as opool,
            tc.tile_pool(name="ypool", bufs=4) as ypool,
        ):
            xT_sb = wpool.tile([128, CCH * N], BF16, tag="xT")
            wqk_sb = wpool.tile([128, CCH * HPC * 128], BF16, tag="wqk")
            wv_sb = wpool.tile([128, CCH * HPC * 64], BF16, tag="wv")
            wp_sb = wpool.tile([64, HPC * DIM], BF16, tag="wp")
            # per head: cols 0:4096 k^T (both partition halves), 4096:8192 q^T
            T = [
                qkpool.tile([128, 2 * N], BF16, tag=f"T{h}", name=f"T{h}")
                for h in range(HPC)
            ]
            # per head: 32 blocks of [128 tok, 64 v | 1 ones]
            V = [
                qkpool.tile([128, NKB * 65], BF16, tag=f"V{h}", name=f"V{h}")
                for h in range(HPC)
            ]

            for c in range(CCH):
                nc.sync.dma_start(
                    out=xT_sb[:, c * N:(c + 1) * N], in_=xT[c * 128:(c + 1) * 128, :]
                )
                nc.sync.dma_start(
                    out=wqk_sb[:, c * 384:(c + 1) * 384],
                    in_=wqkT[c * 128:(c + 1) * 128, :],
                )
                nc.sync.dma_start(
                    out=wv_sb[:, c * 192:(c + 1) * 192],
                    in_=wvT[c * 128:(c + 1) * 128, :],
                )
            for h in range(HPC):
                nc.sync.dma_start(
                    out=wp_sb[0:64, h * DIM:(h + 1) * DIM],
                    in_=wpT[h * 64:(h + 1) * 64, :],
                )
                nc.gpsimd.memset(V[h][:], 1.0)

            # ---- Phase A: qkv projections ----
            def emit_qk_head(h):
                for qt in range(NQT):
                    ps = pspool.tile([128, QT], F32, tag="s")
                    for c in range(CCH):
                        nc.tensor.matmul(
                            ps[:],
                            lhsT=wqk_sb[:, c * 384 + h * 128: c * 384 + (h + 1) * 128],
                            rhs=xT_sb[:, c * N + qt * QT: c * N + (qt + 1) * QT],
                            start=(c == 0),
                            stop=(c == CCH - 1),
                        )
                    # q rows 0:64 -> q^T lo copy; k rows 64:128 -> k^T hi copy
                    nc.vector.tensor_copy(
                        T[h][0:64, N + qt * QT: N + (qt + 1) * QT], ps[0:64, :]
                    )
                    nc.vector.tensor_copy(
                        T[h][64:128, qt * QT:(qt + 1) * QT], ps[64:128, :]
                    )
                # duplicate to the other partition half (cross-partition: DMA)
                nc.sync.dma_start(out=T[h][0:64, 0:N], in_=T[h][64:128, 0:N])
                nc.sync.dma_start(out=T[h][64:128, N:2 * N], in_=T[h][0:64, N:2 * N])

            def emit_v_all():
                for tt in range(NKB):
                    ps = vpool.tile([128, HPC * 64], F32, tag="v")
                    for c in range(CCH):
                        nc.tensor.matmul(
                            ps[:],
                            lhsT=xT_sb[:, c * N + tt * 128: c * N + tt * 128 + 128],
                            rhs=wv_sb[:, c * 192:(c + 1) * 192],
                            start=(c == 0),
                            stop=(c == CCH - 1),
                        )
                    for h in range(HPC):
                        nc.vector.tensor_copy(
                            V[h][:, tt * 65: tt * 65 + 64],
                            ps[:, h * 64:(h + 1) * 64],
                        )

            # ---- Phase B: attention per head (emitted head-major so ACT can
            # start as soon as head 0's q/k are ready); proj deferred ----
            ngrp = (NKB + GRP - 1) // GRP
            O = [[None] * HPC for _ in range(NQT)]

            ess_store = {}

            # S^T pairs + exps are emitted separately from AV so S pairs stay
            # adjacent in PE issue order (even kb -> array rows 0-63, odd kb ->
            # rows 64-127, concurrent). AV matmuls are emitted later (lower
            # priority) and fill PE stalls via buffer pressure.
            def emit_sexp(h, qt):
                ess = []
                for g in range(ngrp):
                    nkb = min(GRP, NKB - g * GRP)
                    ps = pspool.tile(
                        [128, nkb * QT], F32, tag="s", name=f"ps{h}_{qt}_{g}"
                    )
                    es = espool.tile(
                        [128, nkb * QT], BF16, tag="es", name=f"es{h}_{qt}_{g}"
                    )
                    for j in range(nkb):
                        kb = g * GRP + j
                        o = 64 * (kb % 2)
                        nc.tensor.matmul(
                            ps[:, j * QT:(j + 1) * QT],
                            lhsT=T[h][o:o + 64, kb * KB:(kb + 1) * KB],
                            rhs=T[h][o:o + 64, N + qt * QT: N + (qt + 1) * QT],
                            start=True,
                            stop=True,
                        )
                    nc.scalar.activation(
                        es[:], ps[:], mybir.ActivationFunctionType.Exp, scale=SCALE
                    )
                    ess.append((es, nkb))
                ess_store[(h, qt)] = ess

            def emit_av_norm(h, qt):
                po = accpool.tile([65, QT], F32, tag="o", name=f"po{h}_{qt}")
                for g, (es, nkb) in enumerate(ess_store[(h, qt)]):
                    for j in range(nkb):
                        kb = g * GRP + j
                        nc.tensor.matmul(
                            po[:],
                            lhsT=V[h][:, kb * 65: kb * 65 + 65],
                            rhs=es[:, j * QT:(j + 1) * QT],
                            start=(kb == 0),
                            stop=(kb == NKB - 1),
                            skip_group_check=True,
                        )
                # normalize: O^T[0:64] / den(row 64)
                # NB: reciprocal_approx_fast misreads PSUM sources on HW —
                # copy the denominator row to SBUF first.
                dr0 = dpool.tile([1, QT], F32, tag="dr0", name=f"dr0_{h}_{qt}")
                nc.vector.tensor_copy(dr0[:], po[64:65, :])
                dr = dpool.tile([1, QT], F32, tag="dr", name=f"dr{h}_{qt}")
                nc.vector.reciprocal_approx_fast(out=dr[:], in_=dr0[:])
                db = dpool.tile([64, QT], F32, tag="db", name=f"db{h}_{qt}")
                nc.gpsimd.partition_broadcast(db[:], dr[:])
                oh = opool.tile([64, QT], BF16, tag="O", name=f"O{h}_{qt}")
                nc.vector.tensor_mul(oh[:], po[0:64, :], db[:])
                O[qt][h] = oh

            # head 0: S+exp can start as soon as qk_h0 is done; v (needed by
            # AV) is emitted after so it runs in PE slack but before AV deps.
            emit_qk_head(0)
            for qt in range(NQT):
                emit_sexp(0, qt)
            emit_v_all()
            for qt in range(NQT):
                emit_av_norm(0, qt)
            for h in (1, 2):
                emit_qk_head(h)
                for qt in range(NQT):
                    emit_sexp(h, qt)
                    emit_av_norm(h, qt)

            # ---- Phase C: output projection ----
            for qt in range(NQT):
                for oc in range(CCH):
                    py = accpool.tile([128, QT], F32, tag="yb", name=f"py{qt}_{oc}")
                    for h in range(HPC):
                        nc.tensor.matmul(
                            py[:],
                            lhsT=wp_sb[0:64, h * DIM + oc * 128: h * DIM + (oc + 1) * 128],
                            rhs=O[qt][h][:],
                            start=(h == 0),
                            stop=(h == HPC - 1),
                        )
                    ysb = ypool.tile([128, QT], F32, tag="y", name=f"y{qt}_{oc}")
                    nc.vector.tensor_copy(ysb[:], py[:])
                    nc.sync.dma_start(
                        out=yT[oc * 128:(oc + 1) * 128, qt * QT:(qt + 1) * QT],
                        in_=ysb[:],
                    )

    nc.compile()
    return nc


def make_in_maps(x, w_qkv):
    """Build the 8 per-core input maps from the full fp32 inputs."""
    bf = ml_dtypes.bfloat16
    in_maps = []
    for core in range(NCORES):
        b = core // 4
        hs = [(core % 4) * HPC + i for i in range(HPC)]
        xTb = np.ascontiguousarray(np.asarray(x[b]).T).astype(bf)
        wqk = np.empty((DIM, HPC * 128), dtype=bf)
        wv = np.empty((DIM, HPC * 64), dtype=bf)
        for i, h in enumerate(hs):
            wqk[:, i * 128: i * 128 + 64] = w_qkv[h * 64:(h + 1) * 64, :].T
            wqk[:, i * 128 + 64: i * 128 + 128] = w_qkv[DIM + h * 64: DIM + (h + 1) * 64, :].T
            wv[:, i * 64:(i + 1) * 64] = w_qkv[2 * DIM + h * 64: 2 * DIM + (h + 1) * 64, :].T
        in_maps.append({"xT": xTb, "wqkT": wqk, "wvT": wv})
    return in_maps


def make_wp_map(core, w_proj):
    bf = ml_dtypes.bfloat16
    hs = [(core % 4) * HPC + i for i in range(HPC)]
    wp = np.empty((HPC * 64, DIM), dtype=bf)
    for i, h in enumerate(hs):
        wp[i * 64:(i + 1) * 64, :] = w_proj[:, h * 64:(h + 1) * 64].T
    return wp


_NC = None


def kernel(x, w_qkv, w_proj, b_proj):
    global _NC
    if _NC is None:
        _NC = build_program()
    x = np.asarray(x, dtype=np.float32)
    w_qkv = np.asarray(w_qkv, dtype=np.float32)
    w_proj = np.asarray(w_proj, dtype=np.float32)
    b_proj = np.asarray(b_proj, dtype=np.float32)

    in_maps = make_in_maps(x, w_qkv)
    for core in range(NCORES):
        in_maps[core]["wpT"] = make_wp_map(core, w_proj)

    r = run_bass_kernel_spmd(_NC, in_maps, list(range(NCORES)))
    y = np.zeros((B, N, DIM), dtype=np.float32)
    for core in range(NCORES):
        b = core // 4
        y[b] += r.results[core]["yT"].T
    y += b_proj[None, None, :]
    return y


# revision 12
# speedup vs baseline: 1.0444x; 1.0423x over previous
"""Trainium2 Bass kernel for nn_Attention (B=2, N=4096, DIM=768, H=12 heads).

Sharding: 24 (batch, head) pairs over 8 cores -> 3 heads per core, 4 cores
per batch element. Each core computes, for its batch b and its 3 heads:
  q,k,v projections -> full attention (flash-style, no score materialization
  to HBM) -> partial output projection  y_partial^T = sum_h wp_h^T @ O_h^T.
The host sums the 4 partials per batch and adds the bias.

Device dataflow (all matmul inputs bf16, fp32 PSUM accumulation):
  - host passes x^T, w_qk^T, w_v^T, w_p^T pre-transposed/pre-sliced in bf16
  - qk^T = [w_q|w_k]^T.T @ x^T    -> q^T,k^T [64, 4096] per head (d-major)
  - S^T[k,q] = k^T.T @ q^T        -> PSUM, 2-way row-packed (K=64)
  - P^T = exp(S^T * scale)        -> ScalarE (the bottleneck engine)
  - O^T|den = [V|1].T @ P^T       -> PSUM accumulate over k blocks
  - O^T /= den (recip + gpsimd partition-broadcast + DVE mult)
  - y^T += wp_h^T.T @ O_h^T       -> per-head K=64 accumulation
"""

import numpy as np
import ml_dtypes

import concourse.bacc as bacc
import concourse.mybir as mybir
import concourse.tile as tile
from concourse.bass_utils import run_bass_kernel_spmd

BF16 = mybir.dt.bfloat16
F32 = mybir.dt.float32

DIM = 768
N = 4096
NUM_HEADS = 12
HEAD_DIM = 64
SCALE = HEAD_DIM ** -0.5
B = 2
NCORES = 8
HPC = 3  # heads per core
CCH = DIM // 128  # 6 contraction chunks of 128
NQT = 8  # q tiles of 512
QT = 512
NKB = 32  # k blocks of 128
KB = 128
GRP = 2  # k-blocks per exp group (2 PSUM banks, aligns with row-pack pairs)


def build_program():
    nc = bacc.Bacc("TRN2", target_bir_lowering=False, debug=False)

    xT = nc.dram_tensor("xT", [DIM, N], BF16, kind="ExternalInput")
    wqkT = nc.dram_tensor("wqkT", [DIM, HPC * 128], BF16, kind="ExternalInput")
    wvT = nc.dram_tensor("wvT", [DIM, HPC * 64], BF16, kind="ExternalInput")
    wpT = nc.dram_tensor("wpT", [HPC * 64, DIM], BF16, kind="ExternalInput")
    yT = nc.dram_tensor("yT", [DIM, N], F32, kind="ExternalOutput")

    ngrp = NKB // GRP

    with tile.TileContext(nc) as tc:
        with (
            tc.tile_pool(name="wpool", bufs=1) as wpool,
            tc.tile_pool(name="qkpool", bufs=1) as qkpool,
            tc.tile_pool(name="pspool", bufs=2, space="PSUM") as pspool,
            tc.tile_pool(name="vpool", bufs=1, space="PSUM") as vpool,
            tc.tile_pool(name="accpool", bufs=1, space="PSUM") as accpool,
            tc.tile_pool(name="espool", bufs=18) as espool,
            tc.tile_pool(name="dpool", bufs=3) as dpool,
            tc.tile_pool(name="opool", bufs=19) as opool,
            tc.tile_pool(name="ypool", bufs=4) as ypool,
        ):
            # xT split into one tile per q-token-tile so phase A can start as
            # soon as the first slice lands (per-tile DMA dependencies).
            xTs = [
                wpool.tile([128, CCH * QT], BF16, tag=f"xT{qt}", name=f"xT{qt}")
                for qt in range(NQT)
            ]
            wqk_sb = wpool.tile([128, CCH * HPC * 128], BF16, tag="wqk")
            wv_sb = wpool.tile([128, CCH * HPC * 64], BF16, tag="wv")
            wp_sb = wpool.tile([64, HPC * DIM], BF16, tag="wp")
            T = [
                qkpool.tile([128, 2 * N], BF16, tag=f"T{h}", name=f"T{h}")
                for h in range(HPC)
            ]
            V = [
                qkpool.tile([128, NKB * 65], BF16, tag=f"V{h}", name=f"V{h}")
                for h in range(HPC)
            ]

            for c in range(CCH):
                nc.sync.dma_start(
                    out=wqk_sb[:, c * 384:(c + 1) * 384],
                    in_=wqkT[c * 128:(c + 1) * 128, :],
                )
            for qt in range(NQT):
                for c in range(CCH):
                    nc.sync.dma_start(
                        out=xTs[qt][:, c * QT:(c + 1) * QT],
                        in_=xT[c * 128:(c + 1) * 128, qt * QT:(qt + 1) * QT],
                    )
            for c in range(CCH):
                nc.sync.dma_start(
                    out=wv_sb[:, c * 192:(c + 1) * 192],
                    in_=wvT[c * 128:(c + 1) * 128, :],
                )
            for h in range(HPC):
                nc.sync.dma_start(
                    out=wp_sb[0:64, h * DIM:(h + 1) * DIM],
                    in_=wpT[h * 64:(h + 1) * 64, :],
                )
                nc.gpsimd.memset(V[h][:], 1.0)

            # ---- emission helpers ----
            def emit_qk_tile(h, qt):
                ps = pspool.tile([128, QT], F32, tag="s", name=f"qk{h}_{qt}")
                for c in range(CCH):
                    nc.tensor.matmul(
                        ps[:],
                        lhsT=wqk_sb[:, c * 384 + h * 128: c * 384 + (h + 1) * 128],
                        rhs=xTs[qt][:, c * QT:(c + 1) * QT],
                        start=(c == 0),
                        stop=(c == CCH - 1),
                    )
                nc.vector.tensor_copy(
                    T[h][0:64, N + qt * QT: N + (qt + 1) * QT], ps[0:64, :]
                )
                nc.vector.tensor_copy(
                    T[h][64:128, qt * QT:(qt + 1) * QT], ps[64:128, :]
                )

            def emit_qk_dup(h):
                nc.sync.dma_start(out=T[h][0:64, 0:N], in_=T[h][64:128, 0:N])
                nc.sync.dma_start(out=T[h][64:128, N:2 * N], in_=T[h][0:64, N:2 * N])

            def emit_v_tile(tt):
                ps = vpool.tile([128, HPC * 64], F32, tag="v", name=f"v{tt}")
                for c in range(CCH):
                    nc.tensor.matmul(
                        ps[:],
                        lhsT=xTs[tt // 4][:, c * QT + (tt % 4) * 128: c * QT + (tt % 4) * 128 + 128],
                        rhs=wv_sb[:, c * 192:(c + 1) * 192],
                        start=(c == 0),
                        stop=(c == CCH - 1),
                    )
                for h in range(HPC):
                    nc.vector.tensor_copy(
                        V[h][:, tt * 65: tt * 65 + 64],
                        ps[:, h * 64:(h + 1) * 64],
                    )

            es_store = {}

            def emit_s_group(h, qt, g):
                ps = pspool.tile([128, GRP * QT], F32, tag="s", name=f"ps{h}_{qt}_{g}")
                es = espool.tile([128, GRP * QT], BF16, tag="es", name=f"es{h}_{qt}_{g}")
                for j in range(GRP):
                    kb = g * GRP + j
                    o = 64 * (kb % 2)
                    nc.tensor.matmul(
                        ps[:, j * QT:(j + 1) * QT],
                        lhsT=T[h][o:o + 64, kb * KB:(kb + 1) * KB],
                        rhs=T[h][o:o + 64, N + qt * QT: N + (qt + 1) * QT],
                        start=True,
                        stop=True,
                    )
                nc.scalar.activation(
                    es[:], ps[:], mybir.ActivationFunctionType.Exp, scale=SCALE
                )
                es_store[(h, qt, g)] = es

            def emit_av_group(h, qt, g, po):
                es = es_store.pop((h, qt, g))
                for j in range(GRP):
                    kb = g * GRP + j
                    nc.tensor.matmul(
                        po[:],
                        lhsT=V[h][:, kb * 65: kb * 65 + 65],
                        rhs=es[:, j * QT:(j + 1) * QT],
                        start=(kb == 0),
                        stop=(kb == NKB - 1),
                        skip_group_check=True,
                    )

            O = [[None] * HPC for _ in range(NQT)]

            def emit_norm(h, qt, po):
                # reciprocal_approx_fast misreads PSUM sources on HW — bounce
                # the denominator row through SBUF first.
                dr0 = dpool.tile([1, QT], F32, tag="dr0", name=f"dr0_{h}_{qt}")
                nc.vector.tensor_copy(dr0[:], po[64:65, :])
                dr = dpool.tile([1, QT], F32, tag="dr", name=f"dr{h}_{qt}")
                nc.vector.reciprocal_approx_fast(out=dr[:], in_=dr0[:])
                db = dpool.tile([64, QT], F32, tag="db", name=f"db{h}_{qt}")
                nc.gpsimd.partition_broadcast(db[:], dr[:])
                oh = opool.tile([64, QT], BF16, tag="O", name=f"O{h}_{qt}")
                nc.vector.tensor_mul(oh[:], po[0:64, :], db[:])
                O[qt][h] = oh

            def emit_attn(h, qt, filler=None):
                """Interleaved S/exp/AV for one (head, q-tile); AV lags S by 2
                groups so the PE always has AV work while ACT runs exp."""
                po = accpool.tile([65, QT], F32, tag="o", name=f"po{h}_{qt}")
                for g in range(ngrp):
                    emit_s_group(h, qt, g)
                    if g == 1 and filler is not None:
                        filler()
                    if g >= 2:
                        emit_av_group(h, qt, g - 2, po)
                emit_av_group(h, qt, ngrp - 2, po)
                emit_av_group(h, qt, ngrp - 1, po)
                emit_norm(h, qt, po)

            def emit_proj(qt):
                for oc in range(CCH):
                    py = accpool.tile([128, QT], F32, tag="yb", name=f"py{qt}_{oc}")
                    for h in range(HPC):
                        nc.tensor.matmul(
                            py[:],
                            lhsT=wp_sb[0:64, h * DIM + oc * 128: h * DIM + (oc + 1) * 128],
                            rhs=O[qt][h][:],
                            start=(h == 0),
                            stop=(h == HPC - 1),
                        )
                    ysb = ypool.tile([128, QT], F32, tag="y", name=f"y{qt}_{oc}")
                    nc.vector.tensor_copy(ysb[:], py[:])
                    nc.sync.dma_start(
                        out=yT[oc * 128:(oc + 1) * 128, qt * QT:(qt + 1) * QT],
                        in_=ysb[:],
                    )

            # ---- static schedule ----
            for qt in range(NQT):
                emit_qk_tile(0, qt)
            emit_qk_dup(0)

            # head 0, q-tile 0: S+exp first (feeds ACT early), v while ACT
            # chews, then the deferred AV for qt0.
            po00 = accpool.tile([65, QT], F32, tag="o", name="po0_0")
            for g in range(ngrp):
                emit_s_group(0, 0, g)
            for tt in range(NKB):
                emit_v_tile(tt)
            for g in range(ngrp):
                emit_av_group(0, 0, g, po00)
            emit_norm(0, 0, po00)

            for qt in range(1, NQT):
                filler = (lambda q=qt: emit_qk_tile(1, q - 1)) if qt <= NQT - 1 else None
                emit_attn(0, qt, filler=filler)
            emit_qk_tile(1, NQT - 1)
            emit_qk_dup(1)

            for qt in range(NQT):
                filler = (lambda q=qt: emit_qk_tile(2, q)) if qt < NQT else None
                emit_attn(1, qt, filler=filler)
            emit_qk_dup(2)

            for qt in range(NQT):
                filler = (lambda q=qt: emit_proj(q - 1)) if qt >= 1 else None
                emit_attn(2, qt, filler=filler)
            emit_proj(NQT - 2)
            emit_proj(NQT - 1)

    nc.compile()
    return nc


def make_in_maps(x, w_qkv):
    """Build the 8 per-core input maps from the full fp32 inputs."""
    bf = ml_dtypes.bfloat16
    in_maps = []
    for core in range(NCORES):
        b = core // 4
        hs = [(core % 4) * HPC + i for i in range(HPC)]
        xTb = np.ascontiguousarray(np.asarray(x[b]).T).astype(bf)
        wqk = np.empty((DIM, HPC * 128), dtype=bf)
        wv = np.empty((DIM, HPC * 64), dtype=bf)
        for i, h in enumerate(hs):
            wqk[:, i * 128: i * 128 + 64] = w_qkv[h * 64:(h + 1) * 64, :].T
            wqk[:, i * 128 + 64: i * 128 + 128] = w_qkv[DIM + h * 64: DIM + (h + 1) * 64, :].T
            wv[:, i * 64:(i + 1) * 64] = w_qkv[2 * DIM + h * 64: 2 * DIM + (h + 1) * 64, :].T
        in_maps.append({"xT": xTb, "wqkT": wqk, "wvT": wv})
    return in_maps


def make_wp_map(core, w_proj):
    bf = ml_dtypes.bfloat16
    hs = [(core % 4) * HPC + i for i in range(HPC)]
    wp = np.empty((HPC * 64, DIM), dtype=bf)
    for i, h in enumerate(hs):
        wp[i * 64:(i + 1) * 64, :] = w_proj[:, h * 64:(h + 1) * 64].T
    return wp


_NC = None


def kernel(x, w_qkv, w_proj, b_proj):
    global _NC
    if _NC is None:
        _NC = build_program()
    x = np.asarray(x, dtype=np.float32)
    w_qkv = np.asarray(w_qkv, dtype=np.float32)
    w_proj = np.asarray(w_proj, dtype=np.float32)
    b_proj = np.asarray(b_proj, dtype=np.float32)

    in_maps = make_in_maps(x, w_qkv)
    for core in range(NCORES):
        in_maps[core]["wpT"] = make_wp_map(core, w_proj)

    r = run_bass_kernel_spmd(_NC, in_maps, list(range(NCORES)))
    y = np.zeros((B, N, DIM), dtype=np.float32)
    for core in range(NCORES):
        b = core // 4
        y[b] += r.results[core]["yT"].T
    y += b_proj[None, None, :]
    return y


# revision 14
# speedup vs baseline: 1.1338x; 1.0856x over previous
"""Trainium2 Bass kernel for nn_Attention (B=2, N=4096, DIM=768, H=12 heads).

Sharding: 24 (batch, head) pairs over 8 cores -> 3 heads per core, 4 cores
per batch element. Each core computes, for its batch b and its 3 heads:
  q,k,v projections -> full attention (flash-style, no score materialization
  to HBM) -> partial output projection  y_partial^T = sum_h wp_h^T @ O_h^T.
The host sums the 4 partials per batch and adds the bias.

Device dataflow (all matmul inputs bf16, fp32 PSUM accumulation):
  - host passes x^T, w_qk^T, w_v^T, w_p^T pre-transposed/pre-sliced in bf16
  - qk^T = [w_q|w_k]^T.T @ x^T    -> q^T,k^T [64, 4096] per head (d-major)
  - S^T[k,q] = k^T.T @ q^T        -> PSUM, 2-way row-packed (K=64)
  - P^T = exp(S^T * scale)        -> ScalarE (the bottleneck engine)
  - O^T|den = [V|1].T @ P^T       -> PSUM accumulate over k blocks
  - O^T /= den (recip + gpsimd partition-broadcast + DVE mult)
  - y^T += wp_h^T.T @ O_h^T       -> per-head K=64 accumulation
"""

import numpy as np
import ml_dtypes

import concourse.bacc as bacc
import concourse.mybir as mybir
import concourse.tile as tile
from concourse.bass_utils import run_bass_kernel_spmd

BF16 = mybir.dt.bfloat16
F32 = mybir.dt.float32

DIM = 768
N = 4096
NUM_HEADS = 12
HEAD_DIM = 64
SCALE = HEAD_DIM ** -0.5
B = 2
NCORES = 8
HPC = 3  # heads per core
CCH = DIM // 128  # 6 contraction chunks of 128
NQT = 8  # q tiles of 512
QT = 512
NKB = 32  # k blocks of 128
KB = 128
GRP = 2  # k-blocks per exp group (2 PSUM banks, aligns with row-pack pairs)


def build_program():
    nc = bacc.Bacc("TRN2", target_bir_lowering=False, debug=False)

    xT = nc.dram_tensor("xT", [DIM, N], BF16, kind="ExternalInput")
    wqkT = nc.dram_tensor("wqkT", [DIM, HPC * 128], BF16, kind="ExternalInput")
    wvT = nc.dram_tensor("wvT", [DIM, HPC * 64], BF16, kind="ExternalInput")
    wpT = nc.dram_tensor("wpT", [HPC * 64, DIM], BF16, kind="ExternalInput")
    yT = nc.dram_tensor("yT", [DIM, N], F32, kind="ExternalOutput")

    ngrp = NKB // GRP

    with tile.TileContext(nc) as tc:
        with (
            tc.tile_pool(name="wpool", bufs=1) as wpool,
            tc.tile_pool(name="qkpool", bufs=1) as qkpool,
            tc.tile_pool(name="pspool", bufs=2, space="PSUM") as pspool,
            tc.tile_pool(name="vpool", bufs=1, space="PSUM") as vpool,
            tc.tile_pool(name="accpool", bufs=1, space="PSUM") as accpool,
            tc.tile_pool(name="espool", bufs=18) as espool,
            tc.tile_pool(name="dpool", bufs=3) as dpool,
            tc.tile_pool(name="opool", bufs=19) as opool,
            tc.tile_pool(name="ypool", bufs=4) as ypool,
        ):
            # xT split into one tile per q-token-tile so phase A can start as
            # soon as the first slice lands (per-tile DMA dependencies).
            xTs = [
                wpool.tile([128, CCH * QT], BF16, tag=f"xT{qt}", name=f"xT{qt}")
                for qt in range(NQT)
            ]
            wqk_sb = wpool.tile([128, CCH * HPC * 128], BF16, tag="wqk")
            wv_sb = wpool.tile([128, CCH * HPC * 64], BF16, tag="wv")
            wp_sb = wpool.tile([64, HPC * DIM], BF16, tag="wp")
            T = [
                qkpool.tile([128, 2 * N], BF16, tag=f"T{h}", name=f"T{h}")
                for h in range(HPC)
            ]
            V = [
                qkpool.tile([128, NKB * 65], BF16, tag=f"V{h}", name=f"V{h}")
                for h in range(HPC)
            ]

            # consolidated multi-dim DMAs: one instruction per destination
            # tile keeps the Sync sequencer FIFO short (it issues serially).
            wqk_src = wqkT[:].rearrange("(c p) n -> p c n", p=128)
            nc.sync.dma_start(
                out=wqk_sb[:].rearrange("p (c n) -> p c n", n=384), in_=wqk_src
            )
            xT_src = xT[:].rearrange("(c p) n -> p c n", p=128)
            for qt in range(NQT):
                nc.sync.dma_start(
                    out=xTs[qt][:].rearrange("p (c n) -> p c n", n=QT),
                    in_=xT_src[:, :, qt * QT:(qt + 1) * QT],
                )
            nc.sync.dma_start(
                out=wv_sb[:].rearrange("p (c n) -> p c n", n=192),
                in_=wvT[:].rearrange("(c p) n -> p c n", p=128),
            )
            nc.sync.dma_start(
                out=wp_sb[0:64, :].rearrange("p (h n) -> p h n", n=DIM),
                in_=wpT[:].rearrange("(h p) n -> p h n", p=64),
            )
            for h in range(HPC):
                nc.gpsimd.memset(V[h][:], 1.0)

            # ---- emission helpers ----
            def emit_qk_tile(h, qt):
                ps = pspool.tile([128, QT], F32, tag="s", name=f"qk{h}_{qt}")
                for c in range(CCH):
                    nc.tensor.matmul(
                        ps[:],
                        lhsT=wqk_sb[:, c * 384 + h * 128: c * 384 + (h + 1) * 128],
                        rhs=xTs[qt][:, c * QT:(c + 1) * QT],
                        start=(c == 0),
                        stop=(c == CCH - 1),
                    )
                nc.vector.tensor_copy(
                    T[h][0:64, N + qt * QT: N + (qt + 1) * QT], ps[0:64, :]
                )
                nc.vector.tensor_copy(
                    T[h][64:128, qt * QT:(qt + 1) * QT], ps[64:128, :]
                )

            def emit_qk_dup(h):
                nc.sync.dma_start(out=T[h][0:64, 0:N], in_=T[h][64:128, 0:N])
                nc.sync.dma_start(out=T[h][64:128, N:2 * N], in_=T[h][0:64, N:2 * N])

            def emit_v_tile(tt):
                ps = vpool.tile([128, HPC * 64], F32, tag="v", name=f"v{tt}")
                for c in range(CCH):
                    nc.tensor.matmul(
                        ps[:],
                        lhsT=xTs[tt // 4][:, c * QT + (tt % 4) * 128: c * QT + (tt % 4) * 128 + 128],
                        rhs=wv_sb[:, c * 192:(c + 1) * 192],
                        start=(c == 0),
                        stop=(c == CCH - 1),
                    )
                for h in range(HPC):
                    nc.vector.tensor_copy(
                        V[h][:, tt * 65: tt * 65 + 64],
                        ps[:, h * 64:(h + 1) * 64],
                    )

            es_store = {}

            def emit_s_group(h, qt, g):
                ps = pspool.tile([128, GRP * QT], F32, tag="s", name=f"ps{h}_{qt}_{g}")
                es = espool.tile([128, GRP * QT], BF16, tag="es", name=f"es{h}_{qt}_{g}")
                for j in range(GRP):
                    kb = g * GRP + j
                    o = 64 * (kb % 2)
                    nc.tensor.matmul(
                        ps[:, j * QT:(j + 1) * QT],
                        lhsT=T[h][o:o + 64, kb * KB:(kb + 1) * KB],
                        rhs=T[h][o:o + 64, N + qt * QT: N + (qt + 1) * QT],
                        start=True,
                        stop=True,
                    )
                nc.scalar.activation(
                    es[:], ps[:], mybir.ActivationFunctionType.Exp, scale=SCALE
                )
                es_store[(h, qt, g)] = es

            def emit_av_group(h, qt, g, po):
                es = es_store.pop((h, qt, g))
                for j in range(GRP):
                    kb = g * GRP + j
                    nc.tensor.matmul(
                        po[:],
                        lhsT=V[h][:, kb * 65: kb * 65 + 65],
                        rhs=es[:, j * QT:(j + 1) * QT],
                        start=(kb == 0),
                        stop=(kb == NKB - 1),
                        skip_group_check=True,
                    )

            O = [[None] * HPC for _ in range(NQT)]

            def emit_norm(h, qt, po):
                # reciprocal_approx_fast misreads PSUM sources on HW — bounce
                # the denominator row through SBUF first.
                dr0 = dpool.tile([1, QT], F32, tag="dr0", name=f"dr0_{h}_{qt}")
                nc.vector.tensor_copy(dr0[:], po[64:65, :])
                dr = dpool.tile([1, QT], F32, tag="dr", name=f"dr{h}_{qt}")
                nc.vector.reciprocal_approx_fast(out=dr[:], in_=dr0[:])
                db = dpool.tile([64, QT], F32, tag="db", name=f"db{h}_{qt}")
                nc.gpsimd.partition_broadcast(db[:], dr[:])
                oh = opool.tile([64, QT], BF16, tag="O", name=f"O{h}_{qt}")
                nc.vector.tensor_mul(oh[:], po[0:64, :], db[:])
                O[qt][h] = oh

            def emit_attn(h, qt, filler=None):
                """Interleaved S/exp/AV for one (head, q-tile); AV lags S by 2
                groups so the PE always has AV work while ACT runs exp."""
                po = accpool.tile([65, QT], F32, tag="o", bufs=2, name=f"po{h}_{qt}")
                for g in range(ngrp):
                    emit_s_group(h, qt, g)
                    if g == 1 and filler is not None:
                        filler()
                    if g >= 2:
                        emit_av_group(h, qt, g - 2, po)
                emit_av_group(h, qt, ngrp - 2, po)
                emit_av_group(h, qt, ngrp - 1, po)
                emit_norm(h, qt, po)

            def emit_proj(qt):
                for oc in range(CCH):
                    py = accpool.tile([128, QT], F32, tag="yb", name=f"py{qt}_{oc}")
                    for h in range(HPC):
                        nc.tensor.matmul(
                            py[:],
                            lhsT=wp_sb[0:64, h * DIM + oc * 128: h * DIM + (oc + 1) * 128],
                            rhs=O[qt][h][:],
                            start=(h == 0),
                            stop=(h == HPC - 1),
                        )
                    ysb = ypool.tile([128, QT], F32, tag="y", name=f"y{qt}_{oc}")
                    nc.vector.tensor_copy(ysb[:], py[:])
                    nc.sync.dma_start(
                        out=yT[oc * 128:(oc + 1) * 128, qt * QT:(qt + 1) * QT],
                        in_=ysb[:],
                    )

            # ---- static schedule ----
            for qt in range(NQT):
                emit_qk_tile(0, qt)
            emit_qk_dup(0)

            # head 0, q-tile 0: S+exp first (feeds ACT early), v while ACT
            # chews, then the deferred AV for qt0.
            po00 = accpool.tile([65, QT], F32, tag="o", bufs=2, name="po0_0")
            for g in range(ngrp):
                emit_s_group(0, 0, g)
            for tt in range(NKB):
                emit_v_tile(tt)
            for g in range(ngrp):
                emit_av_group(0, 0, g, po00)
            emit_norm(0, 0, po00)

            def qk_filler(nexth, qt):
                if qt < NQT - 1:
                    return lambda: emit_qk_tile(nexth, qt - 1)

                def last():
                    emit_qk_tile(nexth, NQT - 2)
                    emit_qk_tile(nexth, NQT - 1)
                    emit_qk_dup(nexth)

                return last

            for qt in range(1, NQT):
                emit_attn(0, qt, filler=qk_filler(1, qt))

            for qt in range(NQT):
                filler = qk_filler(2, qt) if qt >= 1 else None
                emit_attn(1, qt, filler=filler)

            for qt in range(NQT):
                filler = (lambda q=qt: emit_proj(q - 1)) if qt >= 1 else None
                emit_attn(2, qt, filler=filler)
            emit_proj(NQT - 2)
            emit_proj(NQT - 1)

    nc.compile()
    return nc


def make_in_maps(x, w_qkv):
    """Build the 8 per-core input maps from the full fp32 inputs."""
    bf = ml_dtypes.bfloat16
    in_maps = []
    for core in range(NCORES):
        b = core // 4
        hs = [(core % 4) * HPC + i for i in range(HPC)]
        xTb = np.ascontiguousarray(np.asarray(x[b]).T).astype(bf)
        wqk = np.empty((DIM, HPC * 128), dtype=bf)
        wv = np.empty((DIM, HPC * 64), dtype=bf)
        for i, h in enumerate(hs):
            wqk[:, i * 128: i * 128 + 64] = w_qkv[h * 64:(h + 1) * 64, :].T
            wqk[:, i * 128 + 64: i * 128 + 128] = w_qkv[DIM + h * 64: DIM + (h + 1) * 64, :].T
            wv[:, i * 64:(i + 1) * 64] = w_qkv[2 * DIM + h * 64: 2 * DIM + (h + 1) * 64, :].T
        in_maps.append({"xT": xTb, "wqkT": wqk, "wvT": wv})
    return in_maps


def make_wp_map(core, w_proj):
    bf = ml_dtypes.bfloat16
    hs = [(core % 4) * HPC + i for i in range(HPC)]
    wp = np.empty((HPC * 64, DIM), dtype=bf)
    for i, h in enumerate(hs):
        wp[i * 64:(i + 1) * 64, :] = w_proj[:, h * 64:(h + 1) * 64].T
    return wp


_NC = None


def kernel(x, w_qkv, w_proj, b_proj):
    global _NC
    if _NC is None:
        _NC = build_program()
    x = np.asarray(x, dtype=np.float32)
    w_qkv = np.asarray(w_qkv, dtype=np.float32)
    w_proj = np.asarray(w_proj, dtype=np.float32)
    b_proj = np.asarray(b_proj, dtype=np.float32)

    in_maps = make_in_maps(x, w_qkv)
    for core in range(NCORES):
        in_maps[core]["wpT"] = make_wp_map(core, w_proj)

    r = run_bass_kernel_spmd(_NC, in_maps, list(range(NCORES)))
    y = np.zeros((B, N, DIM), dtype=np.float32)
    for core in range(NCORES):
        b = core // 4
        y[b] += r.results[core]["yT"].T
    y += b_proj[None, None, :]
    return y
